# revision 5
# baseline (speedup 1.0000x reference)
"""PoseNDF distance-field kernel for 8 Trainium2 NeuronCores.

Computes, for each query pose (batch 1024, 21 joints, unit quaternions):
    dist(n,m) = sum_j arccos(<q_nj, t_mj>) / 2   over 10000 train poses
    out(n)    = mean of the 5 smallest dist(n, :)

Strategy (data-parallel over the query batch, 128 queries/core):
- Block layout: 6 queries x 21 joints = 126 (+2 pad) PSUM partitions.
  A single K=84 matmul with block-diagonal weights computes all 21
  per-joint quaternion dots for 6 queries at once:
      c[(n,j), m] = <q_{6b+n, j}, t_{m, j}>
- arccos in ONE ScalarE pass via a custom activation table: we rebuild
  the ACT spline tables (walrus --act-root-json) so that `Sin` evaluates
  g(z) = arccos(1 - z); the activation's free input affine supplies
  z = 1 - c.  The z exponent ladder absorbs the sqrt singularity at c=1.
- Joint sum via a second "selector" matmul (K=128) with -0.5 weights that
  also packs all query rows of a block into one [128, m] PSUM tile,
  accumulating over the 22 blocks:  dneg[n, m] = -dist(n, m).
- Top-5: VectorE Max8 instruction gives the 8 largest of dneg (= 5
  smallest dist) per partition in one pass over [128, 10000].
"""

import hashlib
import json
import os
import shutil
import struct

import numpy as np

N_Q = 1024
N_T = 10000
N_J = 21
N_CORES = 8
QPC = N_Q // N_CORES          # queries per core (128)
QPB = 6                       # queries per block (6*21 = 126 partitions)
N_BLK = 22                    # 21 full blocks + 1 block of 2 queries
CHUNK = 1024                  # train poses per chunk (2 PSUM banks)
MM_N = 512                    # matmul free dim = 1 PSUM bank (fp32)
# chunk starts: 9 full 1024-chunks + one 784 tail, each split at 512
CHUNK_STARTS = list(range(0, N_T, CHUNK))

_EMIN = -22                   # smallest z exponent with real spline buckets
_SECT_BITS = 5                # 32 sections per exponent group

_CACHE = {}


# ----------------------------------------------------------------------------
# Custom ACT tables: make `Sin` evaluate arccos(1 - z)
# ----------------------------------------------------------------------------

def _f2i(x):
    return struct.unpack("<I", struct.pack("<f", np.float32(x)))[0]


def _fit_cubic(f, lo, hi, n=64):
    k = np.arange(n)
    xs = 0.5 * (lo + hi) + 0.5 * (hi - lo) * np.cos((2 * k + 1) * np.pi / (2 * n))
    c = 0.5 * (lo + hi)
    t = (xs - c).astype(np.float64)
    A = np.stack([np.ones_like(t), t, t * t, t * t * t], axis=1)
    coef, *_ = np.linalg.lstsq(A, f(xs), rcond=None)
    return c, coef


def _g(z):
    return np.arccos(np.clip(1.0 - z, -1.0, 1.0))


def _patch_set(root, set_name):
    """Rewrite `sin` in one activation-table set to evaluate arccos(1-z)."""
    bkt = bytearray(open(f"{root}/{set_name}_bkt.bin", "rb").read())
    ctrl = bytearray(open(f"{root}/{set_name}_ctrl.bin", "rb").read())
    prof = json.load(open(f"{root}/{set_name}.json"))

    n_bkt0 = len(bkt) // 32
    n_ctrl0 = len(ctrl) // 32
    new_bkts, new_ctrls = [], []

    for e in range(_EMIN, 1):          # z in [2^EMIN, 2)
        lo_e = 2.0 ** e
        ns = 1 << _SECT_BITS
        base = n_bkt0 + len(new_bkts)
        lsb = 23 - _SECT_BITS
        new_ctrls.append((_SECT_BITS << 16) | (lsb << 11) | base)
        for s in range(ns):
            c, coef = _fit_cubic(_g, lo_e * (1.0 + s / ns), lo_e * (1.0 + (s + 1) / ns))
            new_bkts.append((coef[0], coef[1], coef[2], coef[3], c))

    zero_b = n_bkt0 + len(new_bkts)
    new_bkts.append((0.0, 0.0, 0.0, 0.0, 0.0))
    pi_b = n_bkt0 + len(new_bkts)
    new_bkts.append((np.pi, 0.0, 0.0, 0.0, 2.0))
    neg_ctrl = n_ctrl0 + len(new_ctrls)
    new_ctrls.append(zero_b)
    ctrl_base_pos = n_ctrl0

    for ent in new_bkts:
        bkt += struct.pack("<5I3I", *(_f2i(v) for v in ent), 0, 0, 0)
    for w in new_ctrls:
        ctrl += struct.pack("<8I", w, 0, 0, 0, 0, 0, 0, 0)

    patched = False
    for ent in prof["profile_meta_data"]:
        if not ent["func_name"].startswith("sin_"):
            continue
        ent.update(
            symmetry_point=0,
            sym_invert_sign_point=0,
            symmetry_opt_en=0,
            symmetry_opt_use_neg_region=0,
            imm_bias=0,
            exp_offset=_EMIN,
            pwl_control_base_pos=ctrl_base_pos,
            pwl_control_base_neg=neg_ctrl,
            small_pos_signal_exp_threshold=127 + _EMIN,
            pos_small_signal_pwl_control=zero_b,
            small_neg_signal_exp_threshold=0,
            neg_small_signal_pwl_control=zero_b,
            large_pos_signal_exp_threshold=128,
            large_pos_signal_mantissa_threshold=0,
            pos_large_signal_pwl_control=pi_b,
            large_neg_signal_exp_threshold=0,
            large_neg_signal_mantissa_threshold=0,
            neg_large_signal_pwl_control=zero_b,
            fnan_result=_f2i(np.nan),
            fpinf_result=_f2i(np.pi),
            fninf_result=0,
            fzero_result=0,
            lower_bound=_f2i(-3.40282e38),
            upper_bound=_f2i(3.40282e38),
        )
        patched = True
    assert patched, f"no sin entry in {set_name}"

    open(f"{root}/{set_name}_bkt.bin", "wb").write(bytes(bkt))
    open(f"{root}/{set_name}_ctrl.bin", "wb").write(bytes(ctrl))
    json.dump(prof, open(f"{root}/{set_name}.json", "w"), indent=1)


def _build_act_root():
    """Copy the stock pwp table dir and patch every set containing `sin`."""
    from neuronxcc.driver.Job import Job
    from neuronxcc.driver.jobs.support.FindActInfo import findActInfoFile

    src = os.path.dirname(findActInfoFile(Job.getPackageDir(), "gen3"))
    tag = f"arccos_v3_{_EMIN}_{_SECT_BITS}"
    dst = os.path.join(os.path.expanduser("~"), ".cache", f"act_root_{tag}")
    marker = os.path.join(dst, "DONE")
    if not os.path.exists(marker):
        if os.path.exists(dst):
            shutil.rmtree(dst)
        shutil.copytree(src, dst)
        os.chmod(dst, 0o755)
        for f in os.listdir(dst):
            os.chmod(os.path.join(dst, f), 0o644)
        info = json.load(open(f"{dst}/act_info.json"))
        for ent in info["act_func_sets"]:
            if "sin" in ent["act"]:
                _patch_set(dst, ent["name"])
        open(marker, "w").write("ok")
    h = hashlib.sha256(tag.encode()).hexdigest()[:8]
    return os.path.join(dst, "act_info.json"), h


# ----------------------------------------------------------------------------
# Host-side input packing
# ----------------------------------------------------------------------------

def _pack_inputs(pose, train_poses):
    q = pose.astype(np.float32)
    q = q / np.linalg.norm(q, axis=-1, keepdims=True)

    tfeat = np.ascontiguousarray(
        train_poses.astype(np.float32).reshape(N_T, N_J * 4).T
    )  # [84, N_T]

    # block-diagonal query weights: qexp[core][b][(j,d), n*21+j] = q[6b+n, j, d]
    qexp = np.zeros((N_CORES, N_BLK, N_J * 4, 128), np.float32)
    # selector weights: selw[b][n*21+j, 6b+n] = -0.5
    selw = np.zeros((N_BLK, 128, 128), np.float32)
    for b in range(N_BLK):
        nq = QPB if b < 21 else QPC - 21 * QPB
        for n in range(nq):
            for j in range(N_J):
                selw[b, n * N_J + j, QPB * b + n] = -0.5
    for r in range(N_CORES):
        qr = q[r * QPC:(r + 1) * QPC]  # [128, 21, 4]
        for b in range(N_BLK):
            nq = QPB if b < 21 else QPC - 21 * QPB
            for n in range(nq):
                for j in range(N_J):
                    qexp[r, b, j * 4:(j + 1) * 4, n * N_J + j] = qr[QPB * b + n, j]
    return qexp, tfeat, selw


# ----------------------------------------------------------------------------
# Device program
# ----------------------------------------------------------------------------

def _build_program(act_hash, reps=1):
    import concourse.bacc as bacc
    import concourse.mybir as mybir
    import concourse.tile as tile

    nc = bacc.Bacc("TRN2", target_bir_lowering=False, debug=False,
                   num_devices=N_CORES)
    f32 = mybir.dt.float32

    qexp_d = nc.dram_tensor("qexp", [N_BLK, N_J * 4, 128], f32, kind="ExternalInput")
    tfeat_d = nc.dram_tensor("tfeat", [N_J * 4, N_T], f32, kind="ExternalInput")
    selw_d = nc.dram_tensor("selw", [N_BLK, 128, 128], f32, kind="ExternalInput")
    # cache-buster: act-table version is part of the program
    ver_d = nc.dram_tensor(f"actv_{act_hash}", [1, 4], f32, kind="ExternalInput")
    out_d = nc.dram_tensor("out", [QPC, 1], f32, kind="ExternalOutput")

    with tile.TileContext(nc) as tc:
        with (
            tc.tile_pool(name="consts", bufs=1) as consts,
            tc.tile_pool(name="theta", bufs=4) as th_pool,
            tc.tile_pool(name="cps", bufs=3, space="PSUM") as c_pool,
            tc.tile_pool(name="dps", bufs=1, space="PSUM") as d_pool,
            tc.tile_pool(name="small", bufs=1) as small,
        ):
            scratch = small.tile([1, 4], f32, tag="scratch")
            nc.sync.dma_start(out=scratch[:], in_=ver_d.ap())

            tfeat = consts.tile([N_J * 4, N_T], f32, tag="tfeat")
            nc.sync.dma_start(out=tfeat[:], in_=tfeat_d.ap())
            qexp_t, selw_t = [], []
            for b in range(N_BLK):
                qt = consts.tile([N_J * 4, 128], f32, tag=f"qexp{b}")
                nc.sync.dma_start(out=qt[:], in_=qexp_d.ap()[b])
                qexp_t.append(qt)
                st = consts.tile([128, 128], f32, tag=f"selw{b}")
                nc.sync.dma_start(out=st[:], in_=selw_d.ap()[b])
                selw_t.append(st)

            dist = consts.tile([128, N_T], f32, tag="dist")

            for _rep in range(reps):
              for k0 in CHUNK_STARTS:
                cw = min(CHUNK, N_T - k0)  # 1024 or 784 for the tail
                dneg = d_pool.tile([128, CHUNK], f32, tag="dneg")
                c = None
                for b in range(N_BLK):
                    c = c_pool.tile([128, CHUNK], f32, tag="c")
                    th = th_pool.tile([128, CHUNK], f32, tag="th")
                    for s0 in range(0, cw, MM_N):
                        sw = min(MM_N, cw - s0)
                        nc.tensor.matmul(
                            c[:, s0:s0 + sw], qexp_t[b][:],
                            tfeat[:, k0 + s0: k0 + s0 + sw],
                            start=True, stop=True,
                        )
                    # theta = arccos(c) via hijacked Sin table: g(1 - c)
                    nc.scalar.activation(
                        th[:, 0:cw], c[:, 0:cw],
                        mybir.ActivationFunctionType.Sin,
                        bias=1.0, scale=-1.0,
                    )
                    for s0 in range(0, cw, MM_N):
                        sw = min(MM_N, cw - s0)
                        nc.tensor.matmul(
                            dneg[:, s0:s0 + sw], selw_t[b][:], th[:, s0:s0 + sw],
                            start=(b == 0), stop=(b == N_BLK - 1),
                        )
                nc.vector.tensor_copy(dist[:, k0:k0 + cw], dneg[:, 0:cw])

            top8 = small.tile([128, 8], f32, tag="top8")
            nc.vector.max(out=top8[:], in_=dist[:])
            s5 = small.tile([128, 1], f32, tag="s5")
            nc.vector.tensor_reduce(
                s5[:], top8[:, 0:5], axis=mybir.AxisListType.X,
                op=mybir.AluOpType.add,
            )
            outt = small.tile([128, 1], f32, tag="outt")
            nc.vector.tensor_scalar(
                out=outt[:], in0=s5[:], scalar1=-0.2, scalar2=None,
                op0=mybir.AluOpType.mult,
            )
            nc.sync.dma_start(out=out_d.ap(), in_=outt[:])

    nc.compile()
    return nc


def _get_compiled():
    if "nc" not in _CACHE:
        act_info_path, act_hash = _build_act_root()
        os.environ["BASS_ACT_ROOT_JSON_PATH"] = act_info_path
        _CACHE["nc"] = _build_program(act_hash)
        _CACHE["act_hash"] = act_hash
    return _CACHE["nc"], _CACHE["act_hash"]


# ----------------------------------------------------------------------------
# Entry point
# ----------------------------------------------------------------------------

def kernel(pose, train_poses):
    from concourse.bass_utils import run_bass_kernel_spmd

    assert pose.shape == (N_Q, N_J, 4) and train_poses.shape == (N_T, N_J, 4)
    nc, act_hash = _get_compiled()
    qexp, tfeat, selw = _pack_inputs(np.asarray(pose), np.asarray(train_poses))

    ver = np.zeros((1, 4), np.float32)
    in_maps = [
        {"qexp": qexp[r], "tfeat": tfeat, "selw": selw, f"actv_{act_hash}": ver}
        for r in range(N_CORES)
    ]
    res = run_bass_kernel_spmd(nc, in_maps, list(range(N_CORES)))
    out = np.concatenate(
        [res.results[r]["out"].reshape(QPC) for r in range(N_CORES)]
    )
    return out.astype(np.float32)


# revision 11
# speedup vs baseline: 135.4021x; 135.4021x over previous
"""PoseNDF distance-field kernel for 8 Trainium2 NeuronCores.

Computes, for each query pose (batch 1024, 21 joints, unit quaternions):
    dist(n,m) = sum_j arccos(<q_nj, t_mj>) / 2   over 10000 train poses
    out(n)    = mean of the 5 smallest dist(n, :)

Strategy (data-parallel over the query batch, 128 queries/core):
- Block layout: 6 queries x 21 joints = 126 (+2 pad) PSUM partitions.
  A single K=84 matmul with block-diagonal weights computes all 21
  per-joint quaternion dots for 6 queries at once:
      c[(n,j), m] = <q_{6b+n, j}, t_{m, j}>
- arccos in ONE ScalarE pass via a custom activation table: we rebuild
  the ACT spline tables (walrus --act-root-json) so that `Sin` evaluates
  g(z) = arccos(1 - z); the activation's free input affine supplies
  z = 1 - c.  The z exponent ladder absorbs the sqrt singularity at c=1.
- Joint sum via a second "selector" matmul (K=128) with -0.5 weights that
  also packs all query rows of a block into one [128, m] PSUM tile,
  accumulating over the 22 blocks:  dneg[n, m] = -dist(n, m).
- Top-5: VectorE Max8 instruction gives the 8 largest of dneg (= 5
  smallest dist) per partition in one pass over [128, 10000].
"""

import hashlib
import json
import os
import shutil
import struct

import numpy as np

N_Q = 1024
N_T = 10000
N_J = 21
N_CORES = 8
QPC = N_Q // N_CORES          # queries per core (128)
QPB = 6                       # queries per block (6*21 = 126 partitions)
N_BLK = 22                    # 21 full blocks + 1 block of 2 queries
CHUNK = 1024                  # train poses per chunk (2 PSUM banks)
MM_N = 512                    # matmul free dim = 1 PSUM bank (fp32)
# chunk starts: 9 full 1024-chunks + one 784 tail, each split at 512
CHUNK_STARTS = list(range(0, N_T, CHUNK))

_EMIN = -22                   # smallest z exponent with real spline buckets
_SECT_BITS = 5                # 32 sections per exponent group

_CACHE = {}


# ----------------------------------------------------------------------------
# Custom ACT tables: make `Sin` evaluate arccos(1 - z)
# ----------------------------------------------------------------------------

def _f2i(x):
    return struct.unpack("<I", struct.pack("<f", np.float32(x)))[0]


def _fit_cubic(f, lo, hi, n=64):
    k = np.arange(n)
    xs = 0.5 * (lo + hi) + 0.5 * (hi - lo) * np.cos((2 * k + 1) * np.pi / (2 * n))
    c = 0.5 * (lo + hi)
    t = (xs - c).astype(np.float64)
    A = np.stack([np.ones_like(t), t, t * t, t * t * t], axis=1)
    coef, *_ = np.linalg.lstsq(A, f(xs), rcond=None)
    return c, coef


def _g(z):
    return np.arccos(np.clip(1.0 - z, -1.0, 1.0))


def _patch_set(root, set_name):
    """Rewrite `sin` in one activation-table set to evaluate arccos(1-z)."""
    bkt = bytearray(open(f"{root}/{set_name}_bkt.bin", "rb").read())
    ctrl = bytearray(open(f"{root}/{set_name}_ctrl.bin", "rb").read())
    prof = json.load(open(f"{root}/{set_name}.json"))

    n_bkt0 = len(bkt) // 32
    n_ctrl0 = len(ctrl) // 32
    new_bkts, new_ctrls = [], []

    for e in range(_EMIN, 1):          # z in [2^EMIN, 2)
        lo_e = 2.0 ** e
        ns = 1 << _SECT_BITS
        base = n_bkt0 + len(new_bkts)
        lsb = 23 - _SECT_BITS
        new_ctrls.append((_SECT_BITS << 16) | (lsb << 11) | base)
        for s in range(ns):
            c, coef = _fit_cubic(_g, lo_e * (1.0 + s / ns), lo_e * (1.0 + (s + 1) / ns))
            new_bkts.append((coef[0], coef[1], coef[2], coef[3], c))

    zero_b = n_bkt0 + len(new_bkts)
    new_bkts.append((0.0, 0.0, 0.0, 0.0, 0.0))
    pi_b = n_bkt0 + len(new_bkts)
    new_bkts.append((np.pi, 0.0, 0.0, 0.0, 2.0))
    neg_ctrl = n_ctrl0 + len(new_ctrls)
    new_ctrls.append(zero_b)
    ctrl_base_pos = n_ctrl0

    for ent in new_bkts:
        bkt += struct.pack("<5I3I", *(_f2i(v) for v in ent), 0, 0, 0)
    for w in new_ctrls:
        ctrl += struct.pack("<8I", w, 0, 0, 0, 0, 0, 0, 0)

    patched = False
    for ent in prof["profile_meta_data"]:
        if not ent["func_name"].startswith("sin_"):
            continue
        ent.update(
            symmetry_point=0,
            sym_invert_sign_point=0,
            symmetry_opt_en=0,
            symmetry_opt_use_neg_region=0,
            imm_bias=0,
            exp_offset=_EMIN,
            pwl_control_base_pos=ctrl_base_pos,
            pwl_control_base_neg=neg_ctrl,
            small_pos_signal_exp_threshold=127 + _EMIN,
            pos_small_signal_pwl_control=zero_b,
            small_neg_signal_exp_threshold=0,
            neg_small_signal_pwl_control=zero_b,
            large_pos_signal_exp_threshold=128,
            large_pos_signal_mantissa_threshold=0,
            pos_large_signal_pwl_control=pi_b,
            large_neg_signal_exp_threshold=0,
            large_neg_signal_mantissa_threshold=0,
            neg_large_signal_pwl_control=zero_b,
            fnan_result=_f2i(np.nan),
            fpinf_result=_f2i(np.pi),
            fninf_result=0,
            fzero_result=0,
            lower_bound=_f2i(-3.40282e38),
            upper_bound=_f2i(3.40282e38),
        )
        patched = True
    assert patched, f"no sin entry in {set_name}"

    open(f"{root}/{set_name}_bkt.bin", "wb").write(bytes(bkt))
    open(f"{root}/{set_name}_ctrl.bin", "wb").write(bytes(ctrl))
    json.dump(prof, open(f"{root}/{set_name}.json", "w"), indent=1)


def _build_act_root():
    """Copy the stock pwp table dir and patch every set containing `sin`."""
    from neuronxcc.driver.Job import Job
    from neuronxcc.driver.jobs.support.FindActInfo import findActInfoFile

    src = os.path.dirname(findActInfoFile(Job.getPackageDir(), "gen3"))
    tag = f"arccos_v3_{_EMIN}_{_SECT_BITS}"
    dst = os.path.join(os.path.expanduser("~"), ".cache", f"act_root_{tag}")
    marker = os.path.join(dst, "DONE")
    if not os.path.exists(marker):
        if os.path.exists(dst):
            shutil.rmtree(dst)
        shutil.copytree(src, dst)
        os.chmod(dst, 0o755)
        for f in os.listdir(dst):
            os.chmod(os.path.join(dst, f), 0o644)
        info = json.load(open(f"{dst}/act_info.json"))
        for ent in info["act_func_sets"]:
            if "sin" in ent["act"]:
                _patch_set(dst, ent["name"])
        open(marker, "w").write("ok")
    h = hashlib.sha256(tag.encode()).hexdigest()[:8]
    return os.path.join(dst, "act_info.json"), h


# ----------------------------------------------------------------------------
# Host-side input packing
# ----------------------------------------------------------------------------

def _pack_inputs(pose, train_poses):
    q = pose.astype(np.float32)
    q = q / np.linalg.norm(q, axis=-1, keepdims=True)

    tfeat = np.ascontiguousarray(
        train_poses.astype(np.float32).reshape(N_T, N_J * 4).T
    )  # [84, N_T]

    # block-diagonal query weights: qexp[core][b][(j,d), n*21+j] = q[6b+n, j, d]
    qexp = np.zeros((N_CORES, N_BLK, N_J * 4, 128), np.float32)
    # selector weights: selw[b][n*21+j, 6b+n] = -0.5
    selw = np.zeros((N_BLK, 128, 128), np.float32)
    for b in range(N_BLK):
        nq = QPB if b < 21 else QPC - 21 * QPB
        for n in range(nq):
            for j in range(N_J):
                selw[b, n * N_J + j, QPB * b + n] = -0.5
    for r in range(N_CORES):
        qr = q[r * QPC:(r + 1) * QPC]  # [128, 21, 4]
        for b in range(N_BLK):
            nq = QPB if b < 21 else QPC - 21 * QPB
            for n in range(nq):
                for j in range(N_J):
                    qexp[r, b, j * 4:(j + 1) * 4, n * N_J + j] = qr[QPB * b + n, j]
    return qexp, tfeat, selw


# ----------------------------------------------------------------------------
# Device program
# ----------------------------------------------------------------------------

def _build_program(act_hash, reps=1, variant=""):
    import concourse.bacc as bacc
    import concourse.mybir as mybir
    import concourse.tile as tile

    nc = bacc.Bacc("TRN2", target_bir_lowering=False, debug=False,
                   num_devices=N_CORES)
    f32 = mybir.dt.float32
    f32r = mybir.dt.float32r

    qexp_d = nc.dram_tensor("qexp", [N_BLK, N_J * 4, 128], f32r, kind="ExternalInput")
    tfeat_d = nc.dram_tensor("tfeat", [N_J * 4, N_T], f32r, kind="ExternalInput")
    selw_d = nc.dram_tensor("selw", [N_BLK, 128, 128], f32r, kind="ExternalInput")
    # cache-buster: act-table version (and bench variant) is part of the program
    ver_d = nc.dram_tensor(f"actv_{act_hash}{variant}", [1, 4], f32,
                           kind="ExternalInput")
    out_d = nc.dram_tensor("out", [QPC, 1], f32, kind="ExternalOutput")

    with tile.TileContext(nc) as tc:
        with (
            tc.tile_pool(name="consts", bufs=1) as consts,
            tc.tile_pool(name="theta", bufs=4) as th_pool,
            tc.tile_pool(name="cps", bufs=2, space="PSUM") as c_pool,
            tc.tile_pool(name="dps", bufs=2, space="PSUM") as d_pool,
            tc.tile_pool(name="small", bufs=1) as small,
        ):
            scratch = small.tile([1, 4], f32, tag="scratch")
            nc.sync.dma_start(out=scratch[:], in_=ver_d.ap())

            tfeat = consts.tile([N_J * 4, N_T], f32r, tag="tfeat")
            nc.sync.dma_start(out=tfeat[:], in_=tfeat_d.ap())
            qexp_t, selw_t = [], []
            for b in range(N_BLK):
                qt = consts.tile([N_J * 4, 128], f32r, tag=f"qexp{b}")
                nc.sync.dma_start(out=qt[:], in_=qexp_d.ap()[b])
                qexp_t.append(qt)
                st = consts.tile([128, 128], f32r, tag=f"selw{b}")
                nc.sync.dma_start(out=st[:], in_=selw_d.ap()[b])
                selw_t.append(st)

            dist = consts.tile([128, N_T], f32, tag="dist")

            # Software pipeline: the selector matmul for job i-LAG runs
            # between c-matmuls for job i, so PE never waits on ACT.
            LAG = 2
            for _rep in range(reps):
                jobs = [(k0, b) for k0 in CHUNK_STARTS for b in range(N_BLK)]
                th_q = {}
                dneg_q = {}
                for i in range(len(jobs) + LAG):
                    if i < len(jobs):
                        k0, b = jobs[i]
                        cw = min(CHUNK, N_T - k0)
                        if b == 0:
                            dneg_q[k0] = d_pool.tile(
                                [128, CHUNK], f32, tag="dneg", name="dneg")
                        c = c_pool.tile([128, CHUNK], f32, tag="c")
                        th = th_pool.tile([128, CHUNK], f32r, tag="th")
                        th_q[i] = th
                        for s0 in range(0, cw, MM_N):
                            sw = min(MM_N, cw - s0)
                            nc.tensor.matmul(
                                c[:, s0:s0 + sw], qexp_t[b][:],
                                tfeat[:, k0 + s0: k0 + s0 + sw],
                                start=True, stop=True,
                            )
                        # theta = arccos(c) via hijacked Sin table: g(1 - c)
                        nc.scalar.activation(
                            th[:, 0:cw], c[:, 0:cw],
                            mybir.ActivationFunctionType.Sin,
                            bias=1.0, scale=-1.0,
                        )
                    if i >= LAG:
                        k0, b = jobs[i - LAG]
                        cw = min(CHUNK, N_T - k0)
                        th = th_q.pop(i - LAG)
                        for s0 in range(0, cw, MM_N):
                            sw = min(MM_N, cw - s0)
                            nc.tensor.matmul(
                                dneg_q[k0][:, s0:s0 + sw], selw_t[b][:],
                                th[:, s0:s0 + sw],
                                start=(b == 0), stop=(b == N_BLK - 1),
                            )
                        if b == N_BLK - 1:
                            nc.vector.tensor_copy(
                                dist[:, k0:k0 + cw], dneg_q.pop(k0)[:, 0:cw])

            top8 = small.tile([128, 8], f32, tag="top8")
            nc.vector.max(out=top8[:], in_=dist[:])
            s5 = small.tile([128, 1], f32, tag="s5")
            nc.vector.tensor_reduce(
                s5[:], top8[:, 0:5], axis=mybir.AxisListType.X,
                op=mybir.AluOpType.add,
            )
            outt = small.tile([128, 1], f32, tag="outt")
            nc.vector.tensor_scalar(
                out=outt[:], in0=s5[:], scalar1=-0.2, scalar2=None,
                op0=mybir.AluOpType.mult,
            )
            nc.sync.dma_start(out=out_d.ap(), in_=outt[:])

    nc.compile()
    return nc


def _get_compiled():
    if "nc" not in _CACHE:
        act_info_path, act_hash = _build_act_root()
        os.environ["BASS_ACT_ROOT_JSON_PATH"] = act_info_path
        _CACHE["nc"] = _build_program(act_hash)
        _CACHE["act_hash"] = act_hash
    return _CACHE["nc"], _CACHE["act_hash"]


# ----------------------------------------------------------------------------
# Entry point
# ----------------------------------------------------------------------------

def kernel(pose, train_poses):
    from concourse.bass_utils import run_bass_kernel_spmd

    assert pose.shape == (N_Q, N_J, 4) and train_poses.shape == (N_T, N_J, 4)
    nc, act_hash = _get_compiled()
    qexp, tfeat, selw = _pack_inputs(np.asarray(pose), np.asarray(train_poses))

    ver = np.zeros((1, 4), np.float32)
    in_maps = [
        {"qexp": qexp[r], "tfeat": tfeat, "selw": selw, f"actv_{act_hash}": ver}
        for r in range(N_CORES)
    ]
    res = run_bass_kernel_spmd(nc, in_maps, list(range(N_CORES)))
    out = np.concatenate(
        [res.results[r]["out"].reshape(QPC) for r in range(N_CORES)]
    )
    return out.astype(np.float32)


# revision 12
# speedup vs baseline: 227.4539x; 1.6798x over previous
"""PoseNDF distance-field kernel for 8 Trainium2 NeuronCores.

Computes, for each query pose (batch 1024, 21 joints, unit quaternions):
    dist(n,m) = sum_j arccos(<q_nj, t_mj>) / 2   over 10000 train poses
    out(n)    = mean of the 5 smallest dist(n, :)

Strategy (data-parallel over the query batch, 128 queries/core):
- Block layout: 6 queries x 21 joints = 126 (+2 pad) PSUM partitions.
  A single K=84 matmul with block-diagonal weights computes all 21
  per-joint quaternion dots for 6 queries at once:
      c[(n,j), m] = <q_{6b+n, j}, t_{m, j}>
- arccos in ONE ScalarE pass via a custom activation table: we rebuild
  the ACT spline tables (walrus --act-root-json) so that `Sin` evaluates
  g(z) = arccos(1 - z); the activation's free input affine supplies
  z = 1 - c.  The z exponent ladder absorbs the sqrt singularity at c=1.
- Joint sum via a second "selector" matmul (K=128) with -0.5 weights that
  also packs all query rows of a block into one [128, m] PSUM tile,
  accumulating over the 22 blocks:  dneg[n, m] = -dist(n, m).
- Top-5: VectorE Max8 instruction gives the 8 largest of dneg (= 5
  smallest dist) per partition in one pass over [128, 10000].
"""

import hashlib
import json
import os
import shutil
import struct

import numpy as np

N_Q = 1024
N_T = 10000
N_J = 21
N_CORES = 8
QPC = N_Q // N_CORES          # queries per core (128)
QPB = 6                       # queries per block (6*21 = 126 partitions)
N_BLK = 22                    # 21 full blocks + 1 block of 2 queries
CHUNK = 1024                  # train poses per chunk (2 PSUM banks)
MM_N = 512                    # matmul free dim = 1 PSUM bank (fp32)
# chunk starts: 9 full 1024-chunks + one 784 tail, each split at 512
CHUNK_STARTS = list(range(0, N_T, CHUNK))

_EMIN = -22                   # smallest z exponent with real spline buckets
_SECT_BITS = 5                # 32 sections per exponent group

_CACHE = {}


# ----------------------------------------------------------------------------
# Custom ACT tables: make `Sin` evaluate arccos(1 - z)
# ----------------------------------------------------------------------------

def _f2i(x):
    return struct.unpack("<I", struct.pack("<f", np.float32(x)))[0]


def _fit_cubic(f, lo, hi, n=64):
    k = np.arange(n)
    xs = 0.5 * (lo + hi) + 0.5 * (hi - lo) * np.cos((2 * k + 1) * np.pi / (2 * n))
    c = 0.5 * (lo + hi)
    t = (xs - c).astype(np.float64)
    A = np.stack([np.ones_like(t), t, t * t, t * t * t], axis=1)
    coef, *_ = np.linalg.lstsq(A, f(xs), rcond=None)
    return c, coef


def _g(z):
    return np.arccos(np.clip(1.0 - z, -1.0, 1.0))


def _patch_set(root, set_name):
    """Rewrite `sin` in one activation-table set to evaluate arccos(1-z)."""
    bkt = bytearray(open(f"{root}/{set_name}_bkt.bin", "rb").read())
    ctrl = bytearray(open(f"{root}/{set_name}_ctrl.bin", "rb").read())
    prof = json.load(open(f"{root}/{set_name}.json"))

    n_bkt0 = len(bkt) // 32
    n_ctrl0 = len(ctrl) // 32
    new_bkts, new_ctrls = [], []

    for e in range(_EMIN, 1):          # z in [2^EMIN, 2)
        lo_e = 2.0 ** e
        ns = 1 << _SECT_BITS
        base = n_bkt0 + len(new_bkts)
        lsb = 23 - _SECT_BITS
        new_ctrls.append((_SECT_BITS << 16) | (lsb << 11) | base)
        for s in range(ns):
            c, coef = _fit_cubic(_g, lo_e * (1.0 + s / ns), lo_e * (1.0 + (s + 1) / ns))
            new_bkts.append((coef[0], coef[1], coef[2], coef[3], c))

    zero_b = n_bkt0 + len(new_bkts)
    new_bkts.append((0.0, 0.0, 0.0, 0.0, 0.0))
    pi_b = n_bkt0 + len(new_bkts)
    new_bkts.append((np.pi, 0.0, 0.0, 0.0, 2.0))
    neg_ctrl = n_ctrl0 + len(new_ctrls)
    new_ctrls.append(zero_b)
    ctrl_base_pos = n_ctrl0

    for ent in new_bkts:
        bkt += struct.pack("<5I3I", *(_f2i(v) for v in ent), 0, 0, 0)
    for w in new_ctrls:
        ctrl += struct.pack("<8I", w, 0, 0, 0, 0, 0, 0, 0)

    patched = False
    for ent in prof["profile_meta_data"]:
        if not ent["func_name"].startswith("sin_"):
            continue
        ent.update(
            symmetry_point=0,
            sym_invert_sign_point=0,
            symmetry_opt_en=0,
            symmetry_opt_use_neg_region=0,
            imm_bias=0,
            exp_offset=_EMIN,
            pwl_control_base_pos=ctrl_base_pos,
            pwl_control_base_neg=neg_ctrl,
            small_pos_signal_exp_threshold=127 + _EMIN,
            pos_small_signal_pwl_control=zero_b,
            small_neg_signal_exp_threshold=0,
            neg_small_signal_pwl_control=zero_b,
            large_pos_signal_exp_threshold=128,
            large_pos_signal_mantissa_threshold=0,
            pos_large_signal_pwl_control=pi_b,
            large_neg_signal_exp_threshold=0,
            large_neg_signal_mantissa_threshold=0,
            neg_large_signal_pwl_control=zero_b,
            fnan_result=_f2i(np.nan),
            fpinf_result=_f2i(np.pi),
            fninf_result=0,
            fzero_result=0,
            lower_bound=_f2i(-3.40282e38),
            upper_bound=_f2i(3.40282e38),
        )
        patched = True
    assert patched, f"no sin entry in {set_name}"

    open(f"{root}/{set_name}_bkt.bin", "wb").write(bytes(bkt))
    open(f"{root}/{set_name}_ctrl.bin", "wb").write(bytes(ctrl))
    json.dump(prof, open(f"{root}/{set_name}.json", "w"), indent=1)


def _build_act_root():
    """Copy the stock pwp table dir and patch every set containing `sin`."""
    from neuronxcc.driver.Job import Job
    from neuronxcc.driver.jobs.support.FindActInfo import findActInfoFile

    src = os.path.dirname(findActInfoFile(Job.getPackageDir(), "gen3"))
    tag = f"arccos_v3_{_EMIN}_{_SECT_BITS}"
    dst = os.path.join(os.path.expanduser("~"), ".cache", f"act_root_{tag}")
    marker = os.path.join(dst, "DONE")
    if not os.path.exists(marker):
        if os.path.exists(dst):
            shutil.rmtree(dst)
        shutil.copytree(src, dst)
        os.chmod(dst, 0o755)
        for f in os.listdir(dst):
            os.chmod(os.path.join(dst, f), 0o644)
        info = json.load(open(f"{dst}/act_info.json"))
        for ent in info["act_func_sets"]:
            if "sin" in ent["act"]:
                _patch_set(dst, ent["name"])
        open(marker, "w").write("ok")
    h = hashlib.sha256(tag.encode()).hexdigest()[:8]
    return os.path.join(dst, "act_info.json"), h


# ----------------------------------------------------------------------------
# Host-side input packing
# ----------------------------------------------------------------------------

def _pack_inputs(pose, train_poses):
    q = pose.astype(np.float32)
    q = q / np.linalg.norm(q, axis=-1, keepdims=True)

    tfeat = np.ascontiguousarray(
        train_poses.astype(np.float32).reshape(N_T, N_J * 4).T
    )  # [84, N_T]

    # block-diagonal query weights: qexp[core][b][(j,d), n*21+j] = q[6b+n, j, d]
    qexp = np.zeros((N_CORES, N_BLK, N_J * 4, 128), np.float32)
    # selector weights: selw[b][n*21+j, 6b+n] = -0.5
    selw = np.zeros((N_BLK, 128, 128), np.float32)
    for b in range(N_BLK):
        nq = QPB if b < 21 else QPC - 21 * QPB
        for n in range(nq):
            for j in range(N_J):
                selw[b, n * N_J + j, QPB * b + n] = -0.5
    for r in range(N_CORES):
        qr = q[r * QPC:(r + 1) * QPC]  # [128, 21, 4]
        for b in range(N_BLK):
            nq = QPB if b < 21 else QPC - 21 * QPB
            for n in range(nq):
                for j in range(N_J):
                    qexp[r, b, j * 4:(j + 1) * 4, n * N_J + j] = qr[QPB * b + n, j]
    return qexp, tfeat, selw


# ----------------------------------------------------------------------------
# Device program
# ----------------------------------------------------------------------------

def _build_program(act_hash, reps=1, variant=""):
    import concourse.bacc as bacc
    import concourse.mybir as mybir
    import concourse.tile as tile

    nc = bacc.Bacc("TRN2", target_bir_lowering=False, debug=False,
                   num_devices=N_CORES)
    f32 = mybir.dt.float32
    f32r = mybir.dt.float32r

    qexp_d = nc.dram_tensor("qexp", [N_BLK, N_J * 4, 128], f32r, kind="ExternalInput")
    tfeat_d = nc.dram_tensor("tfeat", [N_J * 4, N_T], f32r, kind="ExternalInput")
    selw_d = nc.dram_tensor("selw", [N_BLK, 128, 128], f32r, kind="ExternalInput")
    # cache-buster: act-table version (and bench variant) is part of the program
    ver_d = nc.dram_tensor(f"actv_{act_hash}{variant}", [1, 4], f32,
                           kind="ExternalInput")
    out_d = nc.dram_tensor("out", [QPC, 1], f32, kind="ExternalOutput")

    with tile.TileContext(nc) as tc:
        with (
            tc.tile_pool(name="consts", bufs=1) as consts,
            tc.tile_pool(name="tf", bufs=3) as tf_pool,
            tc.tile_pool(name="cps", bufs=2, space="PSUM") as c_pool,
            tc.tile_pool(name="dps", bufs=2, space="PSUM") as d_pool,
            tc.tile_pool(name="small", bufs=1) as small,
        ):
            scratch = small.tile([1, 4], f32, tag="scratch")
            nc.sync.dma_start(out=scratch[:], in_=ver_d.ap())

            qexp_t, selw_t = [], []
            for b in range(N_BLK):
                qt = consts.tile([N_J * 4, 128], f32r, tag=f"qexp{b}")
                nc.sync.dma_start(out=qt[:], in_=qexp_d.ap()[b])
                qexp_t.append(qt)
                st = consts.tile([128, 128], f32r, tag=f"selw{b}")
                nc.sync.dma_start(out=st[:], in_=selw_d.ap()[b])
                selw_t.append(st)

            dist = consts.tile([128, N_T], f32, tag="dist")
            # all 22 blocks' theta for one m-stripe, resident in SBUF
            theta = consts.tile([128, N_BLK * CHUNK], f32r, tag="theta")

            for _rep in range(reps):
                for k0 in CHUNK_STARTS:
                    cw = min(CHUNK, N_T - k0)
                    tf = tf_pool.tile([N_J * 4, CHUNK], f32r, tag="tf")
                    nc.sync.dma_start(out=tf[:, 0:cw],
                                      in_=tfeat_d.ap()[:, k0:k0 + cw])
                    # phase A: dots + arccos for all blocks of this stripe
                    for b in range(N_BLK):
                        c = c_pool.tile([128, CHUNK], f32, tag="c")
                        for s0 in range(0, cw, MM_N):
                            sw = min(MM_N, cw - s0)
                            nc.tensor.matmul(
                                c[:, s0:s0 + sw], qexp_t[b][:],
                                tf[:, s0:s0 + sw],
                                start=True, stop=True,
                            )
                        # theta = arccos(c) via hijacked Sin table: g(1 - c)
                        nc.scalar.activation(
                            theta[:, b * CHUNK: b * CHUNK + cw], c[:, 0:cw],
                            mybir.ActivationFunctionType.Sin,
                            bias=1.0, scale=-1.0,
                        )
                    # phase B: joint-sum selector, accumulating over blocks
                    dneg = d_pool.tile([128, CHUNK], f32, tag="dneg")
                    for b in range(N_BLK):
                        for s0 in range(0, cw, MM_N):
                            sw = min(MM_N, cw - s0)
                            nc.tensor.matmul(
                                dneg[:, s0:s0 + sw], selw_t[b][:],
                                theta[:, b * CHUNK + s0: b * CHUNK + s0 + sw],
                                start=(b == 0), stop=(b == N_BLK - 1),
                            )
                    nc.vector.tensor_copy(dist[:, k0:k0 + cw], dneg[:, 0:cw])

            top8 = small.tile([128, 8], f32, tag="top8")
            nc.vector.max(out=top8[:], in_=dist[:])
            s5 = small.tile([128, 1], f32, tag="s5")
            nc.vector.tensor_reduce(
                s5[:], top8[:, 0:5], axis=mybir.AxisListType.X,
                op=mybir.AluOpType.add,
            )
            outt = small.tile([128, 1], f32, tag="outt")
            nc.vector.tensor_scalar(
                out=outt[:], in0=s5[:], scalar1=-0.2, scalar2=None,
                op0=mybir.AluOpType.mult,
            )
            nc.sync.dma_start(out=out_d.ap(), in_=outt[:])

    nc.compile()
    return nc


def _get_compiled():
    if "nc" not in _CACHE:
        act_info_path, act_hash = _build_act_root()
        os.environ["BASS_ACT_ROOT_JSON_PATH"] = act_info_path
        _CACHE["nc"] = _build_program(act_hash)
        _CACHE["act_hash"] = act_hash
    return _CACHE["nc"], _CACHE["act_hash"]


# ----------------------------------------------------------------------------
# Entry point
# ----------------------------------------------------------------------------

def kernel(pose, train_poses):
    from concourse.bass_utils import run_bass_kernel_spmd

    assert pose.shape == (N_Q, N_J, 4) and train_poses.shape == (N_T, N_J, 4)
    nc, act_hash = _get_compiled()
    qexp, tfeat, selw = _pack_inputs(np.asarray(pose), np.asarray(train_poses))

    ver = np.zeros((1, 4), np.float32)
    in_maps = [
        {"qexp": qexp[r], "tfeat": tfeat, "selw": selw, f"actv_{act_hash}": ver}
        for r in range(N_CORES)
    ]
    res = run_bass_kernel_spmd(nc, in_maps, list(range(N_CORES)))
    out = np.concatenate(
        [res.results[r]["out"].reshape(QPC) for r in range(N_CORES)]
    )
    return out.astype(np.float32)
